# revision 3
# baseline (speedup 1.0000x reference)
"""Trainium2 Bass kernel: per-sample masked conv2d via 1-D Winograd F(2,3).

out[b] = conv2d(x[b], weight * m[b], stride=1, pad=1) + bias

Data parallel over batch (32 -> 8 cores x 4).  The 3x3 conv is decomposed
with 1-D Winograd F(2,3) along H (kw stays a shifted-matmul accumulation):

  per output row-pair t (28 tiles):   d_a = xpad[2t+a],  a = 0..3
    V0 = d0-d2, V1 = d1+d2, V2 = d2-d1, V3 = d1-d3          (input transform)
    U0 = g0, U1 = (g0+g1+g2)/2, U2 = (g0-g1+g2)/2, U3 = g2  (weight transform)
    M_j = sum_{kw,i} U_j  (x)kw  V_j                         (4 j-matmuls)
    out[2t]   = M0 + M1 + M2 + bias
    out[2t+1] = M1 - M2 - M3 + bias                          (inverse)

This trades 18 matmul-passes per output pixel for 12 (1.5x less PE time).
The whole matmul path runs in bf16 (same 1 cycle/row PE rate as f32r, but
~6-cycle inter-matmul turnaround vs ~55, half-cost LDWEIGHTS, and 2x DVE
throughput for the transforms).  The 1/2 G-transform scales AND the bias
fold into the Activation-engine PSUM drains (bias rides M1), so the DVE
inverse writes f32 output directly.  m/x/weight are pre-cast to bf16 on
the host, halving HBM traffic.

Startup is descriptor/HBM-bound: the first matmul can only fire once
wT+mask+x have landed.  m and weight are laid out oc-MAJOR in DRAM so the
kernel streams only the oc0 halves (294KB each) before the first matmul;
x loads are split at row 33 (the t<16 V-transform half) so V tiles for
stripes 0-1 depend on a 472KB chunk, not the full 786KB.  All non-critical
loads (oc1 mask/weight halves, x second chunks) are deferred past the
critical window with tc.tile_wait_until (DMA engines round-robin
descriptors across ALL queued transfers, so any early trigger steals
critical-window bandwidth).  Sample-0 V-tile edge memsets ride GpSimd so
the DVE queue reaches the first V subtract immediately, and the mask
multiply is split 3-way (kh0 / kh1 / kh2) with j-order (0,3,1,2) so the
j0/j3 stationaries unblock the PE before the U combos finish.
"""

import sys
from contextlib import ExitStack

for _p in ("/opt/trn_rl_repo",):
    if _p not in sys.path:
        sys.path.append(_p)

import ml_dtypes
import numpy as np

import concourse.bass as bass
import concourse.mybir as mybir
import concourse.tile as tile
from concourse import bacc, bass_utils

B, FIN, FOUT, KK, H, W = 32, 256, 256, 3, 56, 56
N_CORES = 8
BPC = B // N_CORES          # samples per core = 4
P = 128
NI = FIN // P               # 2
NO = FOUT // P              # 2
NT = H // 2                 # 28 row-pair tiles
KSQ = KK * KK
OCF = KSQ * P               # 1152 free elems per (icc, oc) mask block
# r-tile stripes: (8,8,8,4) measured best in both the f32r era (where
# 392-free was LDWEIGHTS-bound) and the bf16 era (equal (7,7,7,7) stripes
# re-tested ~3.5us worse despite identical total PE cycles)
STRIPES = ((0, 8), (8, 16), (16, 24), (24, 28))
JORD = (0, 3, 1, 2)         # j0/j3 stationaries ready before U combos
XSPL = 33                   # x row split: rows 0..32 cover the t<16 ops
F32 = mybir.dt.float32
BF16 = mybir.dt.bfloat16


def build_program():
    nc = bacc.Bacc("TRN2", target_bir_lowering=False, debug=False,
                   num_devices=N_CORES)

    x_d = nc.dram_tensor("x", [BPC, FIN, H, W], BF16,
                         kind="ExternalInput").ap()
    mt_d = nc.dram_tensor("mt", [BPC, NI, NO, P, OCF], BF16,
                          kind="ExternalInput").ap()
    wt_d = nc.dram_tensor("wt", [NI, NO, P, OCF], BF16,
                          kind="ExternalInput").ap()
    b_d = nc.dram_tensor("bias", [FOUT], F32, kind="ExternalInput").ap()
    o_d = nc.dram_tensor("out", [BPC, FOUT, H, W], F32,
                         kind="ExternalOutput").ap()

    with tile.TileContext(nc) as tc, ExitStack() as ctx:
        consts = ctx.enter_context(tc.tile_pool(name="consts", bufs=1))
        mt_pool = ctx.enter_context(tc.tile_pool(name="mt_pool", bufs=6))
        mw_pool = ctx.enter_context(tc.tile_pool(name="mw_pool", bufs=6))
        u_pool = ctx.enter_context(tc.tile_pool(name="u_pool", bufs=6))
        t_pool = ctx.enter_context(tc.tile_pool(name="t_pool", bufs=2))
        xs_pool = ctx.enter_context(tc.tile_pool(name="xs_pool", bufs=2))
        v_pool = ctx.enter_context(tc.tile_pool(name="v_pool", bufs=18))
        m_pool = ctx.enter_context(tc.tile_pool(name="m_pool", bufs=16))
        of_pool = ctx.enter_context(tc.tile_pool(name="of_pool", bufs=2))
        acc_psum = ctx.enter_context(tc.tile_pool(name="acc_psum", bufs=8,
                                                  space="PSUM"))

        w_tiles = {}
        for icc in range(NI):
            for oc in range(NO):
                w_tiles[(icc, oc)] = consts.tile(
                    [P, OCF], BF16, name=f"wt_{icc}_{oc}", tag=f"w{icc}{oc}")

        bias_t = consts.tile([P, NO], F32, name="bias_t")
        # zero scratch row for the V0 t=0 edge case
        zeros = consts.tile([P, W], BF16, name="zeros")
        nc.vector.memset(zeros, 0.0)

        x_nat = x_d.rearrange("s (c p) h w -> s c p (h w)", p=P)
        o_nat = o_d.rearrange("s (c p) h w -> s c p (h w)", p=P)

        mt_tiles = {}
        xs_tiles = {}
        stat = {}     # (s, icc, oc, j) -> [stationary APs per kw]
        vt = {}       # (s, icc) -> [V_j tiles]

        def load_mt(s, icc, oc, ring=None):
            ring = ring or nc.sync
            mt = mt_pool.tile([P, OCF], BF16, name=f"mt_{s}_{icc}_{oc}",
                              tag="mt")
            ring.dma_start(out=mt, in_=mt_d[s, icc, oc])
            mt_tiles[(s, icc, oc)] = mt

        def u_build(s, icc, oc, split3=False):
            # mw = (weight*m) for this oc: [128, (kh kw) * 128]
            mw = mw_pool.tile([P, KSQ * P], BF16,
                              name=f"mw_{s}_{icc}_{oc}", tag="mw")
            mtv = mt_tiles[(s, icc, oc)].rearrange("p (k c) -> p k c", c=P)
            wtv = w_tiles[(icc, oc)].rearrange("p (k c) -> p k c", c=P)
            mwv = mw.rearrange("p (k c) -> p k c", c=P)
            # 3-way split on the critical sample: kh0 unblocks the j0
            # stationary, kh2 unblocks j3, before the combos run
            cuts = ((0, 3), (6, KSQ), (3, 6)) if split3 else ((0, 4), (4, KSQ))
            for k0, k1 in cuts:
                nc.vector.tensor_mul(
                    mwv[:, k0:k1], mtv[:, k0:k1], wtv[:, k0:k1])
            # U combos along kh: mw layout is kh-major [3, 3*128]
            mw3 = mw.rearrange("p (kh r) -> p kh r", kh=KK)
            tt = t_pool.tile([P, KK * P], BF16, name=f"t_{s}_{icc}_{oc}",
                             tag="tt")
            ut = u_pool.tile([P, 2 * KK * P], BF16,
                             name=f"u_{s}_{icc}_{oc}", tag="ut")
            nc.vector.tensor_add(tt, mw3[:, 0], mw3[:, 2])
            nc.vector.tensor_add(ut[:, :KK * P], tt, mw3[:, 1])
            nc.vector.tensor_sub(ut[:, KK * P:], tt, mw3[:, 1])
            bases = (mw[:, :KK * P], ut[:, :KK * P],
                     ut[:, KK * P:], mw[:, 2 * KK * P:])
            for j in range(4):
                stat[(s, icc, oc, j)] = [bases[j][:, kw * P:(kw + 1) * P]
                                         for kw in range(KK)]

        def load_x(s, icc, ring=None, split=False):
            ring = ring or nc.gpsimd
            xs = xs_pool.tile([P, H * W], BF16, name=f"xs_{s}_{icc}",
                              tag="xs")
            if split:
                ring.dma_start(out=xs[:, :XSPL * W],
                               in_=x_nat[s, icc][:, :XSPL * W])
            else:
                ring.dma_start(out=xs, in_=x_nat[s, icc])
            xs_tiles[(s, icc)] = xs

        def load_x_rest(s, icc, ring=None):
            ring = ring or nc.scalar
            ring.dma_start(out=xs_tiles[(s, icc)][:, XSPL * W:],
                           in_=x_nat[s, icc][:, XSPL * W:])

        def v_build(s, icc, edge_engine=None):
            eng = edge_engine or nc.vector
            xsr = xs_tiles[(s, icc)].rearrange("p (t two w) -> p t two w",
                                               two=2, w=W)
            xse = xsr[:, :, 0, :]        # x[2t]
            xso = xsr[:, :, 1, :]        # x[2t+1]
            zrow = zeros[:, 0:W].rearrange("p (o w) -> p o w", o=1)
            vs = []
            for j in range(4):
                v = v_pool.tile([P, NT, W + 2], BF16,
                                name=f"v_{s}_{icc}_{j}", tag="v")
                eng.memset(v[:, :, 0:1], 0.0)
                eng.memset(v[:, :, W + 1:W + 2], 0.0)
                vs.append(v)
            # ops split at t=16 (stripe-2 boundary) so stripes 1-2
            # matmuls unblock on the first-half ops; V0 first since the
            # matmul j-loop consumes j=0 earliest
            HNT = 16
            # V0 = d0-d2 = x[2t-1]-x[2t+1]; t=0 row: 0 - x[1]
            nc.vector.tensor_sub(vs[0][:, 0:1, 1:W + 1],
                                 zrow, xso[:, 0:1, :])
            nc.vector.tensor_sub(vs[0][:, 1:HNT, 1:W + 1],
                                 xso[:, 0:HNT - 1, :], xso[:, 1:HNT, :])
            # V1 = d1+d2 = x[2t] + x[2t+1]; V2 = d2-d1
            nc.vector.tensor_add(vs[1][:, :HNT, 1:W + 1],
                                 xse[:, :HNT, :], xso[:, :HNT, :])
            nc.vector.tensor_sub(vs[2][:, :HNT, 1:W + 1],
                                 xso[:, :HNT, :], xse[:, :HNT, :])
            # V3 = d1-d3 = x[2t]-x[2t+2]
            nc.vector.tensor_sub(vs[3][:, 0:HNT, 1:W + 1],
                                 xse[:, 0:HNT, :], xse[:, 1:HNT + 1, :])
            # second halves (stripes 3-4)
            nc.vector.tensor_sub(vs[0][:, HNT:NT, 1:W + 1],
                                 xso[:, HNT - 1:NT - 1, :],
                                 xso[:, HNT:NT, :])
            nc.vector.tensor_add(vs[1][:, HNT:, 1:W + 1],
                                 xse[:, HNT:, :], xso[:, HNT:, :])
            nc.vector.tensor_sub(vs[2][:, HNT:, 1:W + 1],
                                 xso[:, HNT:, :], xse[:, HNT:, :])
            # V3 t=27 row: x[54] (d3 is the zero pad row)
            nc.vector.tensor_sub(vs[3][:, HNT:NT - 1, 1:W + 1],
                                 xse[:, HNT:NT - 1, :],
                                 xse[:, HNT + 1:NT, :])
            nc.vector.tensor_copy(vs[3][:, NT - 1:NT, 1:W + 1],
                                  xse[:, NT - 1:NT, :])
            vt[(s, icc)] = vs

        def compute_oc(s, oc):
            # bias folds into the M1 drain: exactly one bias reaches each
            # output phase (even = M0+M1'+M2', odd = M1'-M2'-M3 with
            # M1' = 0.5*raw + bias, M2' = 0.5*raw), so the inverse can
            # write f32 output directly -- no separate Act bias pass
            of = of_pool.tile([P, H * W], F32, name=f"of_{s}_{oc}", tag="of")
            ofr = of.rearrange("p (t two w) -> p t two w", two=2, w=W)
            for (t0, t1) in STRIPES:
                stw = (t1 - t0) * W
                accs = [acc_psum.tile([P, stw], F32,
                                      name=f"acc_{s}_{oc}_{t0}_{j}",
                                      tag="acc")
                        for j in range(4)]
                for icc in range(NI):
                    for kw in range(KK):
                        first = (icc == 0 and kw == 0)
                        last = (icc == NI - 1 and kw == KK - 1)
                        for j in JORD:
                            rhs = vt[(s, icc)][j][:, t0:t1, kw:kw + W]
                            nc.tensor.matmul(
                                accs[j], stat[(s, icc, oc, j)][kw], rhs,
                                start=first, stop=last)
                ms = []
                for j in range(4):
                    mj = m_pool.tile([P, stw], BF16,
                                     name=f"m_{s}_{oc}_{t0}_{j}", tag="m")
                    if j == 1:
                        nc.scalar.activation(
                            mj, accs[j], mybir.ActivationFunctionType.Identity,
                            bias=bias_t[:, oc:oc + 1], scale=0.5)
                    elif j == 2:
                        nc.scalar.mul(mj, accs[j], 0.5)
                    else:
                        nc.scalar.copy(mj, accs[j])
                    ms.append(mj)
                # inverse on DVE; the phase-writing ops emit f32 directly
                tmp = t_pool.tile([P, stw], BF16, name=f"it_{s}_{oc}_{t0}",
                                  tag="it")
                msr = [m.rearrange("p (t w) -> p t w", w=W) for m in ms]
                tmpr = tmp.rearrange("p (t w) -> p t w", w=W)
                nc.vector.tensor_add(tmpr, msr[0], msr[1])
                nc.vector.tensor_add(ofr[:, t0:t1, 0, :], tmpr, msr[2])
                nc.vector.tensor_sub(tmpr, msr[1], msr[2])
                nc.vector.tensor_sub(ofr[:, t0:t1, 1, :], tmpr, msr[3])
                # store this stripe (scalar HWDGE ring)
                lo, hi = t0 * 2 * W, t1 * 2 * W
                nc.scalar.dma_start(out=o_nat[s, oc][:, lo:hi],
                                    in_=of[:, lo:hi])

        # --- sample 0 prologue.  Strictly critical-first: the sync ring
        # carries the oc0 mask/weight halves in consumption order, the
        # scalar ring carries the x first-chunks.  Everything else is
        # deferred past the critical window (the DMA engines round-robin
        # ALL queued transfers, so an early trigger steals bandwidth). ---
        nc.sync.dma_start(out=w_tiles[(0, 0)], in_=wt_d[0, 0])
        load_mt(0, 0, 0)
        load_x(0, 0, ring=nc.scalar, split=True)
        nc.sync.dma_start(out=w_tiles[(1, 0)], in_=wt_d[1, 0])
        load_mt(0, 1, 0)
        load_x(0, 1, ring=nc.scalar, split=True)
        # deferred semi-critical: x second chunks (needed by stripe 3,
        # ~5us after the first matmul).  Emitted BEFORE the v_builds that
        # read them -- dependency tracking follows emission order.
        with tc.tile_wait_until(0.006):
            load_x_rest(0, 0, ring=nc.scalar)
            load_x_rest(0, 1, ring=nc.scalar)
        u_build(0, 0, 0, split3=True)
        v_build(0, 0, edge_engine=nc.gpsimd)
        u_build(0, 1, 0, split3=True)
        v_build(0, 1, edge_engine=nc.gpsimd)
        # deferred non-critical: oc1 mask/weight halves (needed ~17us in)
        with tc.tile_wait_until(0.010):
            nc.scalar.dma_start(out=w_tiles[(0, 1)], in_=wt_d[0, 1])
            nc.scalar.dma_start(out=w_tiles[(1, 1)], in_=wt_d[1, 1])
            load_mt(0, 0, 1, ring=nc.scalar)
            load_mt(0, 1, 1, ring=nc.scalar)
        u_build(0, 0, 1)
        u_build(0, 1, 1)
        # bias: 4B-per-descriptor storm -> idle gpsimd ring, out of the way
        nc.gpsimd.dma_start(out=bias_t,
                            in_=b_d.rearrange("(c p) -> p c", p=P))

        # --- software-pipelined emission: the next sample's ic0 prep sits
        # between this sample's oc0 and oc1 so its first stationaries and V
        # tiles are ready on the DVE before the sample boundary ---
        for s in range(BPC):
            compute_oc(s, 0)
            if s + 1 < BPC:
                load_mt(s + 1, 0, 0)
                load_mt(s + 1, 1, 0)
                u_build(s + 1, 0, 0)
                load_x(s + 1, 0)
                v_build(s + 1, 0, edge_engine=nc.gpsimd)
            compute_oc(s, 1)
            if s + 1 < BPC:
                u_build(s + 1, 1, 0)
                load_x(s + 1, 1)
                v_build(s + 1, 1, edge_engine=nc.gpsimd)
                load_mt(s + 1, 0, 1)
                load_mt(s + 1, 1, 1)
                u_build(s + 1, 0, 1)
                u_build(s + 1, 1, 1)

    nc.compile()
    return nc


def shard_inputs(x, m, weight, bias):
    x = np.ascontiguousarray(
        np.asarray(x, dtype=np.float32)).astype(ml_dtypes.bfloat16)
    m = np.asarray(m, dtype=np.float32)
    weight = np.asarray(weight, dtype=np.float32)
    bias = np.ascontiguousarray(np.asarray(bias, dtype=np.float32))
    # oc-major mask layout: [B, NI, NO, P_fin, (kh kw o_in)]
    mt = np.ascontiguousarray(
        m.reshape(B, NO, P, NI, P, KK, KK).transpose(0, 3, 1, 4, 5, 6, 2)
    ).reshape(B, NI, NO, P, OCF).astype(ml_dtypes.bfloat16)
    wt = np.ascontiguousarray(
        weight.reshape(NO, P, NI, P, KK, KK).transpose(2, 0, 3, 4, 5, 1)
    ).reshape(NI, NO, P, OCF).astype(ml_dtypes.bfloat16)
    in_maps = []
    for c in range(N_CORES):
        sl = slice(c * BPC, (c + 1) * BPC)
        in_maps.append({"x": x[sl], "mt": mt[sl], "wt": wt, "bias": bias})
    return in_maps


def kernel(x, m, weight, bias, _trace=False):
    nc = build_program()
    in_maps = shard_inputs(x, m, weight, bias)
    res = bass_utils.run_bass_kernel_spmd(
        nc, in_maps, core_ids=list(range(N_CORES)), trace=_trace
    )
    out = np.concatenate([res.results[c]["out"] for c in range(N_CORES)], axis=0)
    if _trace:
        kernel.last_results = res
    return out


# revision 5
# speedup vs baseline: 1.0020x; 1.0020x over previous
"""Trainium2 Bass kernel: per-sample masked conv2d via 1-D Winograd F(2,3).

out[b] = conv2d(x[b], weight * m[b], stride=1, pad=1) + bias

Data parallel over batch (32 -> 8 cores x 4).  The 3x3 conv is decomposed
with 1-D Winograd F(2,3) along H (kw stays a shifted-matmul accumulation):

  per output row-pair t (28 tiles):   d_a = xpad[2t+a],  a = 0..3
    V0 = d0-d2, V1 = d1+d2, V2 = d2-d1, V3 = d1-d3          (input transform)
    U0 = g0, U1 = (g0+g1+g2)/2, U2 = (g0-g1+g2)/2, U3 = g2  (weight transform)
    M_j = sum_{kw,i} U_j  (x)kw  V_j                         (4 j-matmuls)
    out[2t]   = M0 + M1 + M2 + bias
    out[2t+1] = M1 - M2 - M3 + bias                          (inverse)

This trades 18 matmul-passes per output pixel for 12 (1.5x less PE time).
The whole matmul path runs in bf16 (same 1 cycle/row PE rate as f32r, but
~6-cycle inter-matmul turnaround vs ~55, half-cost LDWEIGHTS, and 2x DVE
throughput for the transforms).  The 1/2 G-transform scales AND the bias
fold into the Activation-engine PSUM drains (bias rides M1), so the DVE
inverse writes f32 output directly.  m/x/weight are pre-cast to bf16 on
the host, halving HBM traffic.

Startup is descriptor/HBM-bound: the first matmul can only fire once
wT+mask+x have landed.  m and weight are laid out oc-MAJOR in DRAM so the
kernel streams only the oc0 halves (294KB each) before the first matmul;
x loads are split at row 33 (the t<16 V-transform half) so V tiles for
stripes 0-1 depend on a 472KB chunk, not the full 786KB.  All non-critical
loads (oc1 mask/weight halves, x second chunks) are deferred past the
critical window with tc.tile_wait_until (DMA engines round-robin
descriptors across ALL queued transfers, so any early trigger steals
critical-window bandwidth).  Sample-0 V-tile edge memsets ride GpSimd so
the DVE queue reaches the first V subtract immediately, and the mask
multiply is split 3-way (kh0 / kh1 / kh2) with j-order (0,3,1,2) so the
j0/j3 stationaries unblock the PE before the U combos finish.
"""

import sys
from contextlib import ExitStack

for _p in ("/opt/trn_rl_repo",):
    if _p not in sys.path:
        sys.path.append(_p)

import ml_dtypes
import numpy as np

import concourse.bass as bass
import concourse.mybir as mybir
import concourse.tile as tile
from concourse import bacc, bass_utils

B, FIN, FOUT, KK, H, W = 32, 256, 256, 3, 56, 56
N_CORES = 8
BPC = B // N_CORES          # samples per core = 4
P = 128
NI = FIN // P               # 2
NO = FOUT // P              # 2
NT = H // 2                 # 28 row-pair tiles
KSQ = KK * KK
OCF = KSQ * P               # 1152 free elems per (icc, oc) mask block
# r-tile stripes: (8,8,8,4) measured best in both the f32r era (where
# 392-free was LDWEIGHTS-bound) and the bf16 era (equal (7,7,7,7) stripes
# re-tested ~3.5us worse despite identical total PE cycles)
STRIPES = ((0, 8), (8, 16), (16, 24), (24, 28))
JORD = (0, 3, 1, 2)         # j0/j3 stationaries ready before U combos
XSPL = 33                   # x row split: rows 0..32 cover the t<16 ops
F32 = mybir.dt.float32
BF16 = mybir.dt.bfloat16


def build_program():
    nc = bacc.Bacc("TRN2", target_bir_lowering=False, debug=False,
                   num_devices=N_CORES)

    x_d = nc.dram_tensor("x", [BPC, FIN, H, W], BF16,
                         kind="ExternalInput").ap()
    mt_d = nc.dram_tensor("mt", [BPC, NI, NO, P, OCF], BF16,
                          kind="ExternalInput").ap()
    wt_d = nc.dram_tensor("wt", [NI, NO, P, OCF], BF16,
                          kind="ExternalInput").ap()
    b_d = nc.dram_tensor("bias", [FOUT], F32, kind="ExternalInput").ap()
    o_d = nc.dram_tensor("out", [BPC, FOUT, H, W], F32,
                         kind="ExternalOutput").ap()

    with tile.TileContext(nc) as tc, ExitStack() as ctx:
        consts = ctx.enter_context(tc.tile_pool(name="consts", bufs=1))
        mt_pool = ctx.enter_context(tc.tile_pool(name="mt_pool", bufs=6))
        mw_pool = ctx.enter_context(tc.tile_pool(name="mw_pool", bufs=6))
        u_pool = ctx.enter_context(tc.tile_pool(name="u_pool", bufs=6))
        t_pool = ctx.enter_context(tc.tile_pool(name="t_pool", bufs=2))
        xs_pool = ctx.enter_context(tc.tile_pool(name="xs_pool", bufs=2))
        v_pool = ctx.enter_context(tc.tile_pool(name="v_pool", bufs=18))
        m_pool = ctx.enter_context(tc.tile_pool(name="m_pool", bufs=16))
        of_pool = ctx.enter_context(tc.tile_pool(name="of_pool", bufs=2))
        acc_psum = ctx.enter_context(tc.tile_pool(name="acc_psum", bufs=8,
                                                  space="PSUM"))

        w_tiles = {}
        for icc in range(NI):
            for oc in range(NO):
                w_tiles[(icc, oc)] = consts.tile(
                    [P, OCF], BF16, name=f"wt_{icc}_{oc}", tag=f"w{icc}{oc}")

        bias_t = consts.tile([P, NO], F32, name="bias_t")
        # zero scratch row for the V0 t=0 edge case
        zeros = consts.tile([P, W], BF16, name="zeros")
        nc.vector.memset(zeros, 0.0)

        x_nat = x_d.rearrange("s (c p) h w -> s c p (h w)", p=P)
        o_nat = o_d.rearrange("s (c p) h w -> s c p (h w)", p=P)

        mt_tiles = {}
        xs_tiles = {}
        stat = {}     # (s, icc, oc, j) -> [stationary APs per kw]
        vt = {}       # (s, icc) -> [V_j tiles]

        def load_mt(s, icc, oc, ring=None):
            ring = ring or nc.sync
            mt = mt_pool.tile([P, OCF], BF16, name=f"mt_{s}_{icc}_{oc}",
                              tag="mt")
            ring.dma_start(out=mt, in_=mt_d[s, icc, oc])
            mt_tiles[(s, icc, oc)] = mt

        def u_build(s, icc, oc, split3=False):
            # mw = (weight*m) for this oc: [128, (kh kw) * 128]
            mw = mw_pool.tile([P, KSQ * P], BF16,
                              name=f"mw_{s}_{icc}_{oc}", tag="mw")
            mtv = mt_tiles[(s, icc, oc)].rearrange("p (k c) -> p k c", c=P)
            wtv = w_tiles[(icc, oc)].rearrange("p (k c) -> p k c", c=P)
            mwv = mw.rearrange("p (k c) -> p k c", c=P)
            # 3-way split on the critical sample: kh0 unblocks the j0
            # stationary, kh2 unblocks j3, before the combos run
            cuts = ((0, 3), (6, KSQ), (3, 6)) if split3 else ((0, 4), (4, KSQ))
            for k0, k1 in cuts:
                nc.vector.tensor_mul(
                    mwv[:, k0:k1], mtv[:, k0:k1], wtv[:, k0:k1])
            # U combos along kh: mw layout is kh-major [3, 3*128]
            mw3 = mw.rearrange("p (kh r) -> p kh r", kh=KK)
            tt = t_pool.tile([P, KK * P], BF16, name=f"t_{s}_{icc}_{oc}",
                             tag="tt")
            ut = u_pool.tile([P, 2 * KK * P], BF16,
                             name=f"u_{s}_{icc}_{oc}", tag="ut")
            nc.vector.tensor_add(tt, mw3[:, 0], mw3[:, 2])
            nc.vector.tensor_add(ut[:, :KK * P], tt, mw3[:, 1])
            nc.vector.tensor_sub(ut[:, KK * P:], tt, mw3[:, 1])
            bases = (mw[:, :KK * P], ut[:, :KK * P],
                     ut[:, KK * P:], mw[:, 2 * KK * P:])
            for j in range(4):
                stat[(s, icc, oc, j)] = [bases[j][:, kw * P:(kw + 1) * P]
                                         for kw in range(KK)]

        def load_x(s, icc, ring=None, split=False):
            ring = ring or nc.gpsimd
            xs = xs_pool.tile([P, H * W], BF16, name=f"xs_{s}_{icc}",
                              tag="xs")
            if split:
                ring.dma_start(out=xs[:, :XSPL * W],
                               in_=x_nat[s, icc][:, :XSPL * W])
            else:
                ring.dma_start(out=xs, in_=x_nat[s, icc])
            xs_tiles[(s, icc)] = xs

        def load_x_rest(s, icc, ring=None):
            ring = ring or nc.scalar
            ring.dma_start(out=xs_tiles[(s, icc)][:, XSPL * W:],
                           in_=x_nat[s, icc][:, XSPL * W:])

        HNT = 16
        vparts = {}

        def v_build_h1(s, icc, eng=None, edge_engine=None):
            # first-half (t<16) V ops: stripes 0-1 matmuls unblock on
            # these; V0 first since the matmul j-loop consumes j=0 first
            ee = edge_engine or nc.vector
            eng = eng or nc.vector
            xsr = xs_tiles[(s, icc)].rearrange("p (t two w) -> p t two w",
                                               two=2, w=W)
            xse = xsr[:, :, 0, :]        # x[2t]
            xso = xsr[:, :, 1, :]        # x[2t+1]
            zrow = zeros[:, 0:W].rearrange("p (o w) -> p o w", o=1)
            vs = []
            for j in range(4):
                v = v_pool.tile([P, NT, W + 2], BF16,
                                name=f"v_{s}_{icc}_{j}", tag="v")
                ee.memset(v[:, :, 0:1], 0.0)
                ee.memset(v[:, :, W + 1:W + 2], 0.0)
                vs.append(v)
            # V0 = d0-d2 = x[2t-1]-x[2t+1]; t=0 row: 0 - x[1]
            eng.tensor_sub(vs[0][:, 0:1, 1:W + 1], zrow, xso[:, 0:1, :])
            eng.tensor_sub(vs[0][:, 1:HNT, 1:W + 1],
                           xso[:, 0:HNT - 1, :], xso[:, 1:HNT, :])
            # V1 = d1+d2 = x[2t] + x[2t+1]; V2 = d2-d1
            eng.tensor_add(vs[1][:, :HNT, 1:W + 1],
                           xse[:, :HNT, :], xso[:, :HNT, :])
            eng.tensor_sub(vs[2][:, :HNT, 1:W + 1],
                           xso[:, :HNT, :], xse[:, :HNT, :])
            # V3 = d1-d3 = x[2t]-x[2t+2]
            eng.tensor_sub(vs[3][:, 0:HNT, 1:W + 1],
                           xse[:, 0:HNT, :], xse[:, 1:HNT + 1, :])
            vparts[(s, icc)] = (vs, xse, xso)
            vt[(s, icc)] = vs

        def v_build_h2(s, icc):
            # second halves (stripes 3-4)
            vs, xse, xso = vparts.pop((s, icc))
            nc.vector.tensor_sub(vs[0][:, HNT:NT, 1:W + 1],
                                 xso[:, HNT - 1:NT - 1, :],
                                 xso[:, HNT:NT, :])
            nc.vector.tensor_add(vs[1][:, HNT:, 1:W + 1],
                                 xse[:, HNT:, :], xso[:, HNT:, :])
            nc.vector.tensor_sub(vs[2][:, HNT:, 1:W + 1],
                                 xso[:, HNT:, :], xse[:, HNT:, :])
            # V3 t=27 row: x[54] (d3 is the zero pad row)
            nc.vector.tensor_sub(vs[3][:, HNT:NT - 1, 1:W + 1],
                                 xse[:, HNT:NT - 1, :],
                                 xse[:, HNT + 1:NT, :])
            nc.vector.tensor_copy(vs[3][:, NT - 1:NT, 1:W + 1],
                                  xse[:, NT - 1:NT, :])

        def v_build(s, icc, eng=None, edge_engine=None):
            v_build_h1(s, icc, eng=eng, edge_engine=edge_engine)
            v_build_h2(s, icc)

        def compute_oc(s, oc):
            # bias folds into the M1 drain: exactly one bias reaches each
            # output phase (even = M0+M1'+M2', odd = M1'-M2'-M3 with
            # M1' = 0.5*raw + bias, M2' = 0.5*raw), so the inverse can
            # write f32 output directly -- no separate Act bias pass
            of = of_pool.tile([P, H * W], F32, name=f"of_{s}_{oc}", tag="of")
            ofr = of.rearrange("p (t two w) -> p t two w", two=2, w=W)
            for (t0, t1) in STRIPES:
                stw = (t1 - t0) * W
                accs = [acc_psum.tile([P, stw], F32,
                                      name=f"acc_{s}_{oc}_{t0}_{j}",
                                      tag="acc")
                        for j in range(4)]
                for icc in range(NI):
                    for kw in range(KK):
                        first = (icc == 0 and kw == 0)
                        last = (icc == NI - 1 and kw == KK - 1)
                        for j in JORD:
                            rhs = vt[(s, icc)][j][:, t0:t1, kw:kw + W]
                            nc.tensor.matmul(
                                accs[j], stat[(s, icc, oc, j)][kw], rhs,
                                start=first, stop=last)
                ms = []
                for j in range(4):
                    mj = m_pool.tile([P, stw], BF16,
                                     name=f"m_{s}_{oc}_{t0}_{j}", tag="m")
                    if j == 1:
                        nc.scalar.activation(
                            mj, accs[j], mybir.ActivationFunctionType.Identity,
                            bias=bias_t[:, oc:oc + 1], scale=0.5)
                    elif j == 2:
                        nc.scalar.mul(mj, accs[j], 0.5)
                    else:
                        nc.scalar.copy(mj, accs[j])
                    ms.append(mj)
                # inverse on DVE; the phase-writing ops emit f32 directly
                tmp = t_pool.tile([P, stw], BF16, name=f"it_{s}_{oc}_{t0}",
                                  tag="it")
                msr = [m.rearrange("p (t w) -> p t w", w=W) for m in ms]
                tmpr = tmp.rearrange("p (t w) -> p t w", w=W)
                nc.vector.tensor_add(tmpr, msr[0], msr[1])
                nc.vector.tensor_add(ofr[:, t0:t1, 0, :], tmpr, msr[2])
                nc.vector.tensor_sub(tmpr, msr[1], msr[2])
                nc.vector.tensor_sub(ofr[:, t0:t1, 1, :], tmpr, msr[3])
                # store this stripe (scalar HWDGE ring)
                lo, hi = t0 * 2 * W, t1 * 2 * W
                nc.scalar.dma_start(out=o_nat[s, oc][:, lo:hi],
                                    in_=of[:, lo:hi])

        # --- sample 0 prologue.  Strictly critical-first: the sync ring
        # carries the oc0 mask/weight halves in consumption order, the
        # scalar ring carries the x first-chunks.  Everything else is
        # deferred past the critical window (the DMA engines round-robin
        # ALL queued transfers, so an early trigger steals bandwidth). ---
        nc.sync.dma_start(out=w_tiles[(0, 0)], in_=wt_d[0, 0])
        load_mt(0, 0, 0)
        load_x(0, 0, ring=nc.scalar, split=True)
        nc.sync.dma_start(out=w_tiles[(1, 0)], in_=wt_d[1, 0])
        load_mt(0, 1, 0)
        load_x(0, 1, ring=nc.scalar, split=True)
        # deferred semi-critical: x second chunks (needed by stripe 3,
        # ~5us after the first matmul).  Emitted BEFORE the v_builds that
        # read them -- dependency tracking follows emission order.
        with tc.tile_wait_until(0.006):
            load_x_rest(0, 0, ring=nc.scalar)
            load_x_rest(0, 1, ring=nc.scalar)
        # deferred non-critical: oc1 mask/weight halves, on the sync ring
        # (idle after the critical prologue; the scalar ring carries the
        # stripe stores which would queue these out past +30us)
        with tc.tile_wait_until(0.008):
            nc.sync.dma_start(out=w_tiles[(0, 1)], in_=wt_d[0, 1])
            nc.sync.dma_start(out=w_tiles[(1, 1)], in_=wt_d[1, 1])
            load_mt(0, 0, 1, ring=nc.sync)
            load_mt(0, 1, 1, ring=nc.sync)
        # ic0's first-half V ops ride GpSimd so the DVE (busy with the U
        # builds and ic1's V ops) isn't the serial feeder of stripe 0
        u_build(0, 0, 0, split3=True)
        v_build_h1(0, 0, eng=nc.gpsimd, edge_engine=nc.gpsimd)
        u_build(0, 1, 0, split3=True)
        v_build_h1(0, 1, edge_engine=nc.gpsimd)
        v_build_h2(0, 0)
        v_build_h2(0, 1)
        u_build(0, 0, 1)
        u_build(0, 1, 1)
        # bias: 4B-per-descriptor storm -> idle gpsimd ring, out of the way
        nc.gpsimd.dma_start(out=bias_t,
                            in_=b_d.rearrange("(c p) -> p c", p=P))

        # --- software-pipelined emission: the next sample's ic0 prep sits
        # between this sample's oc0 and oc1 so its first stationaries and V
        # tiles are ready on the DVE before the sample boundary ---
        for s in range(BPC):
            compute_oc(s, 0)
            if s + 1 < BPC:
                load_mt(s + 1, 0, 0)
                load_mt(s + 1, 1, 0)
                u_build(s + 1, 0, 0)
                load_x(s + 1, 0)
                v_build(s + 1, 0, edge_engine=nc.gpsimd)
            compute_oc(s, 1)
            if s + 1 < BPC:
                u_build(s + 1, 1, 0)
                load_x(s + 1, 1)
                v_build(s + 1, 1, edge_engine=nc.gpsimd)
                load_mt(s + 1, 0, 1)
                load_mt(s + 1, 1, 1)
                u_build(s + 1, 0, 1)
                u_build(s + 1, 1, 1)

    nc.compile()
    return nc


def shard_inputs(x, m, weight, bias):
    x = np.ascontiguousarray(
        np.asarray(x, dtype=np.float32)).astype(ml_dtypes.bfloat16)
    m = np.asarray(m, dtype=np.float32)
    weight = np.asarray(weight, dtype=np.float32)
    bias = np.ascontiguousarray(np.asarray(bias, dtype=np.float32))
    # oc-major mask layout: [B, NI, NO, P_fin, (kh kw o_in)]
    mt = np.ascontiguousarray(
        m.reshape(B, NO, P, NI, P, KK, KK).transpose(0, 3, 1, 4, 5, 6, 2)
    ).reshape(B, NI, NO, P, OCF).astype(ml_dtypes.bfloat16)
    wt = np.ascontiguousarray(
        weight.reshape(NO, P, NI, P, KK, KK).transpose(2, 0, 3, 4, 5, 1)
    ).reshape(NI, NO, P, OCF).astype(ml_dtypes.bfloat16)
    in_maps = []
    for c in range(N_CORES):
        sl = slice(c * BPC, (c + 1) * BPC)
        in_maps.append({"x": x[sl], "mt": mt[sl], "wt": wt, "bias": bias})
    return in_maps


def kernel(x, m, weight, bias, _trace=False):
    nc = build_program()
    in_maps = shard_inputs(x, m, weight, bias)
    res = bass_utils.run_bass_kernel_spmd(
        nc, in_maps, core_ids=list(range(N_CORES)), trace=_trace
    )
    out = np.concatenate([res.results[c]["out"] for c in range(N_CORES)], axis=0)
    if _trace:
        kernel.last_results = res
    return out


# revision 10
# speedup vs baseline: 1.0190x; 1.0170x over previous
"""Trainium2 Bass kernel: per-sample masked conv2d via 1-D Winograd F(2,3).

out[b] = conv2d(x[b], weight * m[b], stride=1, pad=1) + bias

Data parallel over batch (32 -> 8 cores x 4).  The 3x3 conv is decomposed
with 1-D Winograd F(2,3) along H (kw stays a shifted-matmul accumulation):

  per output row-pair t (28 tiles):   d_a = xpad[2t+a],  a = 0..3
    V0 = d0-d2, V1 = d1+d2, V2 = d2-d1, V3 = d1-d3          (input transform)
    U0 = g0, U1 = (g0+g1+g2)/2, U2 = (g0-g1+g2)/2, U3 = g2  (weight transform)
    M_j = sum_{kw,i} U_j  (x)kw  V_j                         (4 j-matmuls)
    out[2t]   = M0 + M1 + M2 + bias
    out[2t+1] = M1 - M2 - M3 + bias                          (inverse)

This trades 18 matmul-passes per output pixel for 12 (1.5x less PE time).
The whole matmul path runs in bf16 (same 1 cycle/row PE rate as f32r, but
~6-cycle inter-matmul turnaround vs ~55, half-cost LDWEIGHTS, and 2x DVE
throughput for the transforms).  The 1/2 G-transform scales AND the bias
fold into the Activation-engine PSUM drains (bias rides M1), so the DVE
inverse writes f32 output directly.  m/x/weight are pre-cast to bf16 on
the host, halving HBM traffic.

Startup is descriptor/HBM-bound: the first matmul can only fire once
wT+mask+x have landed.  m and weight are laid out oc-MAJOR in DRAM so the
kernel streams only the oc0 halves (294KB each) before the first matmul;
x loads are split at row 33 (the t<16 V-transform half) so V tiles for
stripes 0-1 depend on a 472KB chunk, not the full 786KB.  All non-critical
loads (oc1 mask/weight halves, x second chunks) are deferred past the
critical window with tc.tile_wait_until (DMA engines round-robin
descriptors across ALL queued transfers, so any early trigger steals
critical-window bandwidth).  Sample-0 V-tile edge memsets ride GpSimd so
the DVE queue reaches the first V subtract immediately, and the mask
multiply is split 3-way (kh0 / kh1 / kh2) with j-order (0,3,1,2) so the
j0/j3 stationaries unblock the PE before the U combos finish.
"""

import sys
from contextlib import ExitStack

for _p in ("/opt/trn_rl_repo",):
    if _p not in sys.path:
        sys.path.append(_p)

import ml_dtypes
import numpy as np

import concourse.bass as bass
import concourse.mybir as mybir
import concourse.tile as tile
from concourse import bacc, bass_utils

B, FIN, FOUT, KK, H, W = 32, 256, 256, 3, 56, 56
N_CORES = 8
BPC = B // N_CORES          # samples per core = 4
P = 128
NI = FIN // P               # 2
NO = FOUT // P              # 2
NT = H // 2                 # 28 row-pair tiles
KSQ = KK * KK
OCF = KSQ * P               # 1152 free elems per (icc, oc) mask block
# r-tile stripes: (8,8,8,4) measured best in both the f32r era (where
# 392-free was LDWEIGHTS-bound) and the bf16 era (equal (7,7,7,7) stripes
# re-tested ~3.5us worse despite identical total PE cycles)
STRIPES = ((0, 8), (8, 16), (16, 24), (24, 28))
JORD = (0, 3, 1, 2)         # j0/j3 stationaries ready before U combos
XSPL = 33                   # x row split: rows 0..32 cover the t<16 ops
F32 = mybir.dt.float32
BF16 = mybir.dt.bfloat16


def build_program():
    nc = bacc.Bacc("TRN2", target_bir_lowering=False, debug=False,
                   num_devices=N_CORES)

    x_d = nc.dram_tensor("x", [BPC, FIN, H, W], BF16,
                         kind="ExternalInput").ap()
    mt_d = nc.dram_tensor("mt", [BPC, NI, NO, P, OCF], BF16,
                          kind="ExternalInput").ap()
    wt_d = nc.dram_tensor("wt", [NI, NO, P, OCF], BF16,
                          kind="ExternalInput").ap()
    b_d = nc.dram_tensor("bias", [FOUT], F32, kind="ExternalInput").ap()
    o_d = nc.dram_tensor("out", [BPC, FOUT, H, W], F32,
                         kind="ExternalOutput").ap()

    with tile.TileContext(nc) as tc, ExitStack() as ctx:
        consts = ctx.enter_context(tc.tile_pool(name="consts", bufs=1))
        mt_pool = ctx.enter_context(tc.tile_pool(name="mt_pool", bufs=6))
        mw_pool = ctx.enter_context(tc.tile_pool(name="mw_pool", bufs=6))
        u_pool = ctx.enter_context(tc.tile_pool(name="u_pool", bufs=6))
        t_pool = ctx.enter_context(tc.tile_pool(name="t_pool", bufs=2))
        xs_pool = ctx.enter_context(tc.tile_pool(name="xs_pool", bufs=2))
        v_pool = ctx.enter_context(tc.tile_pool(name="v_pool", bufs=18))
        m_pool = ctx.enter_context(tc.tile_pool(name="m_pool", bufs=16))
        of_pool = ctx.enter_context(tc.tile_pool(name="of_pool", bufs=2))
        acc_psum = ctx.enter_context(tc.tile_pool(name="acc_psum", bufs=8,
                                                  space="PSUM"))

        w_tiles = {}
        for icc in range(NI):
            for oc in range(NO):
                w_tiles[(icc, oc)] = consts.tile(
                    [P, OCF], BF16, name=f"wt_{icc}_{oc}", tag=f"w{icc}{oc}")

        bias_t = consts.tile([P, NO], F32, name="bias_t")
        # zero scratch row for the V0 t=0 edge case
        zeros = consts.tile([P, W], BF16, name="zeros")
        nc.vector.memset(zeros, 0.0)

        x_nat = x_d.rearrange("s (c p) h w -> s c p (h w)", p=P)
        o_nat = o_d.rearrange("s (c p) h w -> s c p (h w)", p=P)

        mt_tiles = {}
        xs_tiles = {}
        stat = {}     # (s, icc, oc, j) -> [stationary APs per kw]
        vt = {}       # (s, icc) -> [V_j tiles]

        def load_mt(s, icc, oc, ring=None, ksplit=False):
            ring = ring or nc.sync
            mt = mt_pool.tile([P, OCF], BF16, name=f"mt_{s}_{icc}_{oc}",
                              tag="mt")
            if ksplit:
                # kh0 chunk first: the j0 stationary (and so the first
                # matmul) only needs k<3 of the mask
                ring.dma_start(out=mt[:, :3 * P], in_=mt_d[s, icc, oc][:, :3 * P])
                ring.dma_start(out=mt[:, 3 * P:], in_=mt_d[s, icc, oc][:, 3 * P:])
            else:
                ring.dma_start(out=mt, in_=mt_d[s, icc, oc])
            mt_tiles[(s, icc, oc)] = mt

        def u_build(s, icc, oc, split3=False):
            # mw = (weight*m) for this oc: [128, (kh kw) * 128]
            mw = mw_pool.tile([P, KSQ * P], BF16,
                              name=f"mw_{s}_{icc}_{oc}", tag="mw")
            mtv = mt_tiles[(s, icc, oc)].rearrange("p (k c) -> p k c", c=P)
            wtv = w_tiles[(icc, oc)].rearrange("p (k c) -> p k c", c=P)
            mwv = mw.rearrange("p (k c) -> p k c", c=P)
            # 3-way split on the critical sample: kh0 unblocks the j0
            # stationary, kh2 unblocks j3, before the combos run
            cuts = ((0, 3), (6, KSQ), (3, 6)) if split3 else ((0, 4), (4, KSQ))
            for k0, k1 in cuts:
                nc.vector.tensor_mul(
                    mwv[:, k0:k1], mtv[:, k0:k1], wtv[:, k0:k1])
            # U combos along kh: mw layout is kh-major [3, 3*128]
            mw3 = mw.rearrange("p (kh r) -> p kh r", kh=KK)
            tt = t_pool.tile([P, KK * P], BF16, name=f"t_{s}_{icc}_{oc}",
                             tag="tt")
            ut = u_pool.tile([P, 2 * KK * P], BF16,
                             name=f"u_{s}_{icc}_{oc}", tag="ut")
            nc.vector.tensor_add(tt, mw3[:, 0], mw3[:, 2])
            nc.vector.tensor_add(ut[:, :KK * P], tt, mw3[:, 1])
            nc.vector.tensor_sub(ut[:, KK * P:], tt, mw3[:, 1])
            bases = (mw[:, :KK * P], ut[:, :KK * P],
                     ut[:, KK * P:], mw[:, 2 * KK * P:])
            for j in range(4):
                stat[(s, icc, oc, j)] = [bases[j][:, kw * P:(kw + 1) * P]
                                         for kw in range(KK)]

        def load_x(s, icc, ring=None, split=False):
            ring = ring or nc.gpsimd
            xs = xs_pool.tile([P, H * W], BF16, name=f"xs_{s}_{icc}",
                              tag="xs")
            if split:
                ring.dma_start(out=xs[:, :XSPL * W],
                               in_=x_nat[s, icc][:, :XSPL * W])
            else:
                ring.dma_start(out=xs, in_=x_nat[s, icc])
            xs_tiles[(s, icc)] = xs

        def load_x_rest(s, icc, ring=None):
            ring = ring or nc.scalar
            ring.dma_start(out=xs_tiles[(s, icc)][:, XSPL * W:],
                           in_=x_nat[s, icc][:, XSPL * W:])

        HNT = 16
        vparts = {}

        def v_build_h1(s, icc, eng=None, edge_engine=None):
            # first-half (t<16) V ops: stripes 0-1 matmuls unblock on
            # these; V0 first since the matmul j-loop consumes j=0 first
            ee = edge_engine or nc.vector
            eng = eng or nc.vector
            xsr = xs_tiles[(s, icc)].rearrange("p (t two w) -> p t two w",
                                               two=2, w=W)
            xse = xsr[:, :, 0, :]        # x[2t]
            xso = xsr[:, :, 1, :]        # x[2t+1]
            zrow = zeros[:, 0:W].rearrange("p (o w) -> p o w", o=1)
            vs = []
            for j in range(4):
                v = v_pool.tile([P, NT, W + 2], BF16,
                                name=f"v_{s}_{icc}_{j}", tag="v")
                ee.memset(v[:, :, 0:1], 0.0)
                ee.memset(v[:, :, W + 1:W + 2], 0.0)
                vs.append(v)
            # V0 = d0-d2 = x[2t-1]-x[2t+1]; t=0 row: 0 - x[1]
            eng.tensor_sub(vs[0][:, 0:1, 1:W + 1], zrow, xso[:, 0:1, :])
            eng.tensor_sub(vs[0][:, 1:HNT, 1:W + 1],
                           xso[:, 0:HNT - 1, :], xso[:, 1:HNT, :])
            # V1 = d1+d2 = x[2t] + x[2t+1]; V2 = d2-d1
            eng.tensor_add(vs[1][:, :HNT, 1:W + 1],
                           xse[:, :HNT, :], xso[:, :HNT, :])
            eng.tensor_sub(vs[2][:, :HNT, 1:W + 1],
                           xso[:, :HNT, :], xse[:, :HNT, :])
            # V3 = d1-d3 = x[2t]-x[2t+2]
            eng.tensor_sub(vs[3][:, 0:HNT, 1:W + 1],
                           xse[:, 0:HNT, :], xse[:, 1:HNT + 1, :])
            vparts[(s, icc)] = (vs, xse, xso)
            vt[(s, icc)] = vs

        def v_build_h2(s, icc):
            # second halves (stripes 3-4)
            vs, xse, xso = vparts.pop((s, icc))
            nc.vector.tensor_sub(vs[0][:, HNT:NT, 1:W + 1],
                                 xso[:, HNT - 1:NT - 1, :],
                                 xso[:, HNT:NT, :])
            nc.vector.tensor_add(vs[1][:, HNT:, 1:W + 1],
                                 xse[:, HNT:, :], xso[:, HNT:, :])
            nc.vector.tensor_sub(vs[2][:, HNT:, 1:W + 1],
                                 xso[:, HNT:, :], xse[:, HNT:, :])
            # V3 t=27 row: x[54] (d3 is the zero pad row)
            nc.vector.tensor_sub(vs[3][:, HNT:NT - 1, 1:W + 1],
                                 xse[:, HNT:NT - 1, :],
                                 xse[:, HNT + 1:NT, :])
            nc.vector.tensor_copy(vs[3][:, NT - 1:NT, 1:W + 1],
                                  xse[:, NT - 1:NT, :])

        def v_build(s, icc, eng=None, edge_engine=None):
            v_build_h1(s, icc, eng=eng, edge_engine=edge_engine)
            v_build_h2(s, icc)

        def compute_oc(s, oc, warmup=False):
            # bias folds into the M1 drain: exactly one bias reaches each
            # output phase (even = M0+M1'+M2', odd = M1'-M2'-M3 with
            # M1' = 0.5*raw + bias, M2' = 0.5*raw), so the inverse can
            # write f32 output directly -- no separate Act bias pass
            of = of_pool.tile([P, H * W], F32, name=f"of_{s}_{oc}", tag="of")
            ofr = of.rearrange("p (t two w) -> p t two w", two=2, w=W)

            def alloc_accs(t0, t1):
                stw = (t1 - t0) * W
                return [acc_psum.tile([P, stw], F32,
                                      name=f"acc_{s}_{oc}_{t0}_{j}",
                                      tag="acc")
                        for j in range(4)]

            def mm(t0, t1, accs, icc, first, last):
                for kw in range(KK):
                    for j in JORD:
                        rhs = vt[(s, icc)][j][:, t0:t1, kw:kw + W]
                        nc.tensor.matmul(
                            accs[j], stat[(s, icc, oc, j)][kw], rhs,
                            start=(first and kw == 0),
                            stop=(last and kw == KK - 1))

            def finish(t0, t1, accs):
                stw = (t1 - t0) * W
                ms = []
                for j in range(4):
                    mj = m_pool.tile([P, stw], BF16,
                                     name=f"m_{s}_{oc}_{t0}_{j}", tag="m")
                    if j == 1:
                        nc.scalar.activation(
                            mj, accs[j], mybir.ActivationFunctionType.Identity,
                            bias=bias_t[:, oc:oc + 1], scale=0.5)
                    elif j == 2:
                        nc.scalar.mul(mj, accs[j], 0.5)
                    else:
                        nc.scalar.copy(mj, accs[j])
                    ms.append(mj)
                # inverse on DVE; the phase-writing ops emit f32 directly
                tmp = t_pool.tile([P, stw], BF16, name=f"it_{s}_{oc}_{t0}",
                                  tag="it")
                msr = [m.rearrange("p (t w) -> p t w", w=W) for m in ms]
                tmpr = tmp.rearrange("p (t w) -> p t w", w=W)
                nc.vector.tensor_add(tmpr, msr[0], msr[1])
                nc.vector.tensor_add(ofr[:, t0:t1, 0, :], tmpr, msr[2])
                nc.vector.tensor_sub(tmpr, msr[1], msr[2])
                nc.vector.tensor_sub(ofr[:, t0:t1, 1, :], tmpr, msr[3])
                # store this stripe (scalar HWDGE ring)
                lo, hi = t0 * 2 * W, t1 * 2 * W
                nc.scalar.dma_start(out=o_nat[s, oc][:, lo:hi],
                                    in_=of[:, lo:hi])

            if warmup:
                # stripe-pair interleave: both stripes' ic0 matmuls run
                # back-to-back (8 PSUM banks) so ic1's mask/x transfers
                # get ~4.6us of streaming grace behind ic0's
                (a0, a1), (b0, b1) = STRIPES[0], STRIPES[1]
                accsA, accsB = alloc_accs(a0, a1), alloc_accs(b0, b1)
                mm(a0, a1, accsA, 0, True, False)
                mm(b0, b1, accsB, 0, True, False)
                mm(a0, a1, accsA, 1, False, True)
                mm(b0, b1, accsB, 1, False, True)
                finish(a0, a1, accsA)
                finish(b0, b1, accsB)
                rest = STRIPES[2:]
            else:
                rest = STRIPES
            for (t0, t1) in rest:
                accs = alloc_accs(t0, t1)
                mm(t0, t1, accs, 0, True, False)
                mm(t0, t1, accs, 1, False, True)
                finish(t0, t1, accs)

        # --- sample 0 prologue.  Strictly critical-first: the sync ring
        # carries the oc0 mask/weight halves in consumption order, the
        # scalar ring carries the x first-chunks.  Everything else is
        # deferred past the critical window (the DMA engines round-robin
        # ALL queued transfers, so an early trigger steals bandwidth). ---
        # kh0 chunks first so the j0 stationary's mask-multiply can run
        # before the rest of the tile lands
        nc.sync.dma_start(out=w_tiles[(0, 0)][:, :3 * P],
                          in_=wt_d[0, 0][:, :3 * P])
        load_mt(0, 0, 0, ksplit=True)
        nc.sync.dma_start(out=w_tiles[(0, 0)][:, 3 * P:],
                          in_=wt_d[0, 0][:, 3 * P:])
        load_x(0, 0, ring=nc.scalar, split=True)
        nc.sync.dma_start(out=w_tiles[(1, 0)], in_=wt_d[1, 0])
        load_mt(0, 1, 0)
        load_x(0, 1, ring=nc.scalar, split=True)
        # deferred semi-critical: x second chunks (needed by stripe 3,
        # ~5us after the first matmul).  Emitted BEFORE the v_builds that
        # read them -- dependency tracking follows emission order.
        with tc.tile_wait_until(0.006):
            load_x_rest(0, 0, ring=nc.scalar)
            load_x_rest(0, 1, ring=nc.scalar)
        # deferred non-critical: oc1 mask/weight halves, on the sync ring
        # (idle after the critical prologue; the scalar ring carries the
        # stripe stores which would queue these out past +30us)
        with tc.tile_wait_until(0.008):
            nc.sync.dma_start(out=w_tiles[(0, 1)], in_=wt_d[0, 1])
            nc.sync.dma_start(out=w_tiles[(1, 1)], in_=wt_d[1, 1])
            load_mt(0, 0, 1, ring=nc.sync)
            load_mt(0, 1, 1, ring=nc.sync)
        u_build(0, 0, 0, split3=True)
        v_build_h1(0, 0, edge_engine=nc.gpsimd)
        u_build(0, 1, 0, split3=True)
        v_build_h1(0, 1, edge_engine=nc.gpsimd)
        v_build_h2(0, 0)
        v_build_h2(0, 1)
        u_build(0, 0, 1)
        u_build(0, 1, 1)
        # bias: 4B-per-descriptor storm -> idle gpsimd ring, out of the way
        nc.gpsimd.dma_start(out=bias_t,
                            in_=b_d.rearrange("(c p) -> p c", p=P))

        # --- software-pipelined emission: the next sample's ic0 prep sits
        # between this sample's oc0 and oc1 so its first stationaries and V
        # tiles are ready on the DVE before the sample boundary ---
        for s in range(BPC):
            compute_oc(s, 0, warmup=(s == 0))
            if s + 1 < BPC:
                load_mt(s + 1, 0, 0)
                load_mt(s + 1, 1, 0)
                u_build(s + 1, 0, 0)
                load_x(s + 1, 0)
                v_build(s + 1, 0, edge_engine=nc.gpsimd)
            compute_oc(s, 1)
            if s + 1 < BPC:
                u_build(s + 1, 1, 0)
                load_x(s + 1, 1)
                v_build(s + 1, 1, edge_engine=nc.gpsimd)
                load_mt(s + 1, 0, 1)
                load_mt(s + 1, 1, 1)
                u_build(s + 1, 0, 1)
                u_build(s + 1, 1, 1)

    nc.compile()
    return nc


def shard_inputs(x, m, weight, bias):
    x = np.ascontiguousarray(
        np.asarray(x, dtype=np.float32)).astype(ml_dtypes.bfloat16)
    m = np.asarray(m, dtype=np.float32)
    weight = np.asarray(weight, dtype=np.float32)
    bias = np.ascontiguousarray(np.asarray(bias, dtype=np.float32))
    # oc-major mask layout: [B, NI, NO, P_fin, (kh kw o_in)]
    mt = np.ascontiguousarray(
        m.reshape(B, NO, P, NI, P, KK, KK).transpose(0, 3, 1, 4, 5, 6, 2)
    ).reshape(B, NI, NO, P, OCF).astype(ml_dtypes.bfloat16)
    wt = np.ascontiguousarray(
        weight.reshape(NO, P, NI, P, KK, KK).transpose(2, 0, 3, 4, 5, 1)
    ).reshape(NI, NO, P, OCF).astype(ml_dtypes.bfloat16)
    in_maps = []
    for c in range(N_CORES):
        sl = slice(c * BPC, (c + 1) * BPC)
        in_maps.append({"x": x[sl], "mt": mt[sl], "wt": wt, "bias": bias})
    return in_maps


def kernel(x, m, weight, bias, _trace=False):
    nc = build_program()
    in_maps = shard_inputs(x, m, weight, bias)
    res = bass_utils.run_bass_kernel_spmd(
        nc, in_maps, core_ids=list(range(N_CORES)), trace=_trace
    )
    out = np.concatenate([res.results[c]["out"] for c in range(N_CORES)], axis=0)
    if _trace:
        kernel.last_results = res
    return out


# revision 12
# speedup vs baseline: 1.0313x; 1.0121x over previous
"""Trainium2 Bass kernel: per-sample masked conv2d via 1-D Winograd F(2,3).

out[b] = conv2d(x[b], weight * m[b], stride=1, pad=1) + bias

Data parallel over batch (32 -> 8 cores x 4).  The 3x3 conv is decomposed
with 1-D Winograd F(2,3) along H (kw stays a shifted-matmul accumulation):

  per output row-pair t (28 tiles):   d_a = xpad[2t+a],  a = 0..3
    V0 = d0-d2, V1 = d1+d2, V2 = d2-d1, V3 = d1-d3          (input transform)
    U0 = g0, U1 = (g0+g1+g2)/2, U2 = (g0-g1+g2)/2, U3 = g2  (weight transform)
    M_j = sum_{kw,i} U_j  (x)kw  V_j                         (4 j-matmuls)
    out[2t]   = M0 + M1 + M2 + bias
    out[2t+1] = M1 - M2 - M3 + bias                          (inverse)

This trades 18 matmul-passes per output pixel for 12 (1.5x less PE time).
The whole matmul path runs in bf16 (same 1 cycle/row PE rate as f32r, but
~6-cycle inter-matmul turnaround vs ~55, half-cost LDWEIGHTS, and 2x DVE
throughput for the transforms).  The 1/2 G-transform scales AND the bias
fold into the Activation-engine PSUM drains (bias rides M1), so the DVE
inverse writes f32 output directly.  m/x/weight are pre-cast to bf16 on
the host, halving HBM traffic.

Startup is descriptor/HBM-bound: the first matmul can only fire once
wT+mask+x have landed.  m and weight are laid out oc-MAJOR in DRAM so the
kernel streams only the oc0 halves (294KB each) before the first matmul;
x loads are split at row 33 (the t<16 V-transform half) so V tiles for
stripes 0-1 depend on a 472KB chunk, not the full 786KB.  All non-critical
loads (oc1 mask/weight halves, x second chunks) are deferred past the
critical window with tc.tile_wait_until (DMA engines round-robin
descriptors across ALL queued transfers, so any early trigger steals
critical-window bandwidth).  Sample-0 V-tile edge memsets ride GpSimd so
the DVE queue reaches the first V subtract immediately, and the mask
multiply is split 3-way (kh0 / kh1 / kh2) with j-order (0,3,1,2) so the
j0/j3 stationaries unblock the PE before the U combos finish.
"""

import sys
from contextlib import ExitStack

for _p in ("/opt/trn_rl_repo",):
    if _p not in sys.path:
        sys.path.append(_p)

import ml_dtypes
import numpy as np

import concourse.bass as bass
import concourse.mybir as mybir
import concourse.tile as tile
from concourse import bacc, bass_utils

B, FIN, FOUT, KK, H, W = 32, 256, 256, 3, 56, 56
N_CORES = 8
BPC = B // N_CORES          # samples per core = 4
P = 128
NI = FIN // P               # 2
NO = FOUT // P              # 2
NT = H // 2                 # 28 row-pair tiles
KSQ = KK * KK
OCF = KSQ * P               # 1152 free elems per (icc, oc) mask block
# r-tile stripes: (8,8,8,4) measured best in both the f32r era (where
# 392-free was LDWEIGHTS-bound) and the bf16 era (equal (7,7,7,7) stripes
# re-tested ~3.5us worse despite identical total PE cycles)
STRIPES = ((0, 8), (8, 16), (16, 24), (24, 28))
JORD = (0, 1, 2, 3)         # matches V/U emission order (in-order queues)
XSPL = 33                   # x row split: rows 0..32 cover the t<16 ops
F32 = mybir.dt.float32
BF16 = mybir.dt.bfloat16


def build_program():
    nc = bacc.Bacc("TRN2", target_bir_lowering=False, debug=False,
                   num_devices=N_CORES)

    x_d = nc.dram_tensor("x", [BPC, FIN, H, W], BF16,
                         kind="ExternalInput").ap()
    mt_d = nc.dram_tensor("mt", [BPC, NI, NO, P, OCF], BF16,
                          kind="ExternalInput").ap()
    wt_d = nc.dram_tensor("wt", [NI, NO, P, OCF], BF16,
                          kind="ExternalInput").ap()
    b_d = nc.dram_tensor("bias", [FOUT], F32, kind="ExternalInput").ap()
    o_d = nc.dram_tensor("out", [BPC, FOUT, H, W], F32,
                         kind="ExternalOutput").ap()

    with tile.TileContext(nc) as tc, ExitStack() as ctx:
        consts = ctx.enter_context(tc.tile_pool(name="consts", bufs=1))
        mt_pool = ctx.enter_context(tc.tile_pool(name="mt_pool", bufs=6))
        mw_pool = ctx.enter_context(tc.tile_pool(name="mw_pool", bufs=6))
        u_pool = ctx.enter_context(tc.tile_pool(name="u_pool", bufs=6))
        t_pool = ctx.enter_context(tc.tile_pool(name="t_pool", bufs=2))
        xs_pool = ctx.enter_context(tc.tile_pool(name="xs_pool", bufs=2))
        v_pool = ctx.enter_context(tc.tile_pool(name="v_pool", bufs=18))
        m_pool = ctx.enter_context(tc.tile_pool(name="m_pool", bufs=16))
        of_pool = ctx.enter_context(tc.tile_pool(name="of_pool", bufs=2))
        acc_psum = ctx.enter_context(tc.tile_pool(name="acc_psum", bufs=8,
                                                  space="PSUM"))

        w_tiles = {}
        for icc in range(NI):
            for oc in range(NO):
                w_tiles[(icc, oc)] = consts.tile(
                    [P, OCF], BF16, name=f"wt_{icc}_{oc}", tag=f"w{icc}{oc}")

        bias_t = consts.tile([P, NO], F32, name="bias_t")
        # zero scratch row for the V0 t=0 edge case
        zeros = consts.tile([P, W], BF16, name="zeros")
        nc.vector.memset(zeros, 0.0)

        x_nat = x_d.rearrange("s (c p) h w -> s c p (h w)", p=P)
        o_nat = o_d.rearrange("s (c p) h w -> s c p (h w)", p=P)

        mt_tiles = {}
        xs_tiles = {}
        stat = {}     # (s, icc, oc, j) -> [stationary APs per kw]
        vt = {}       # (s, icc) -> [V_j tiles]

        def load_mt(s, icc, oc, ring=None, ksplit=False):
            ring = ring or nc.sync
            mt = mt_pool.tile([P, OCF], BF16, name=f"mt_{s}_{icc}_{oc}",
                              tag="mt")
            if ksplit:
                # kh0 chunk first: the j0 stationary (and so the first
                # matmul) only needs k<3 of the mask
                ring.dma_start(out=mt[:, :3 * P], in_=mt_d[s, icc, oc][:, :3 * P])
                ring.dma_start(out=mt[:, 3 * P:], in_=mt_d[s, icc, oc][:, 3 * P:])
            else:
                ring.dma_start(out=mt, in_=mt_d[s, icc, oc])
            mt_tiles[(s, icc, oc)] = mt

        def u_build(s, icc, oc, split3=False):
            # mw = (weight*m) for this oc: [128, (kh kw) * 128]
            mw = mw_pool.tile([P, KSQ * P], BF16,
                              name=f"mw_{s}_{icc}_{oc}", tag="mw")
            mtv = mt_tiles[(s, icc, oc)].rearrange("p (k c) -> p k c", c=P)
            wtv = w_tiles[(icc, oc)].rearrange("p (k c) -> p k c", c=P)
            mwv = mw.rearrange("p (k c) -> p k c", c=P)
            # 3-way split on the critical sample: kh0 unblocks the j0
            # stationary, kh2 unblocks j3, before the combos run
            cuts = ((0, 3), (6, KSQ), (3, 6)) if split3 else ((0, 4), (4, KSQ))
            for k0, k1 in cuts:
                nc.vector.tensor_mul(
                    mwv[:, k0:k1], mtv[:, k0:k1], wtv[:, k0:k1])
            # U combos along kh: mw layout is kh-major [3, 3*128]
            mw3 = mw.rearrange("p (kh r) -> p kh r", kh=KK)
            tt = t_pool.tile([P, KK * P], BF16, name=f"t_{s}_{icc}_{oc}",
                             tag="tt")
            ut = u_pool.tile([P, 2 * KK * P], BF16,
                             name=f"u_{s}_{icc}_{oc}", tag="ut")
            nc.vector.tensor_add(tt, mw3[:, 0], mw3[:, 2])
            nc.vector.tensor_add(ut[:, :KK * P], tt, mw3[:, 1])
            nc.vector.tensor_sub(ut[:, KK * P:], tt, mw3[:, 1])
            bases = (mw[:, :KK * P], ut[:, :KK * P],
                     ut[:, KK * P:], mw[:, 2 * KK * P:])
            for j in range(4):
                stat[(s, icc, oc, j)] = [bases[j][:, kw * P:(kw + 1) * P]
                                         for kw in range(KK)]

        def load_x(s, icc, ring=None, split=False):
            ring = ring or nc.gpsimd
            xs = xs_pool.tile([P, H * W], BF16, name=f"xs_{s}_{icc}",
                              tag="xs")
            if split:
                ring.dma_start(out=xs[:, :XSPL * W],
                               in_=x_nat[s, icc][:, :XSPL * W])
            else:
                ring.dma_start(out=xs, in_=x_nat[s, icc])
            xs_tiles[(s, icc)] = xs

        def load_x_rest(s, icc, ring=None):
            ring = ring or nc.scalar
            ring.dma_start(out=xs_tiles[(s, icc)][:, XSPL * W:],
                           in_=x_nat[s, icc][:, XSPL * W:])

        HNT = 16
        vparts = {}

        def v_build_h1(s, icc, eng=None, edge_engine=None):
            # first-half (t<16) V ops: stripes 0-1 matmuls unblock on
            # these; V0 first since the matmul j-loop consumes j=0 first
            ee = edge_engine or nc.vector
            eng = eng or nc.vector
            xsr = xs_tiles[(s, icc)].rearrange("p (t two w) -> p t two w",
                                               two=2, w=W)
            xse = xsr[:, :, 0, :]        # x[2t]
            xso = xsr[:, :, 1, :]        # x[2t+1]
            zrow = zeros[:, 0:W].rearrange("p (o w) -> p o w", o=1)
            vs = []
            for j in range(4):
                v = v_pool.tile([P, NT, W + 2], BF16,
                                name=f"v_{s}_{icc}_{j}", tag="v")
                ee.memset(v[:, :, 0:1], 0.0)
                ee.memset(v[:, :, W + 1:W + 2], 0.0)
                vs.append(v)
            # V0 = d0-d2 = x[2t-1]-x[2t+1]; t=0 row: 0 - x[1]
            eng.tensor_sub(vs[0][:, 0:1, 1:W + 1], zrow, xso[:, 0:1, :])
            eng.tensor_sub(vs[0][:, 1:HNT, 1:W + 1],
                           xso[:, 0:HNT - 1, :], xso[:, 1:HNT, :])
            # V1 = d1+d2 = x[2t] + x[2t+1]; V2 = d2-d1
            eng.tensor_add(vs[1][:, :HNT, 1:W + 1],
                           xse[:, :HNT, :], xso[:, :HNT, :])
            eng.tensor_sub(vs[2][:, :HNT, 1:W + 1],
                           xso[:, :HNT, :], xse[:, :HNT, :])
            # V3 = d1-d3 = x[2t]-x[2t+2]
            eng.tensor_sub(vs[3][:, 0:HNT, 1:W + 1],
                           xse[:, 0:HNT, :], xse[:, 1:HNT + 1, :])
            vparts[(s, icc)] = (vs, xse, xso)
            vt[(s, icc)] = vs

        def v_build_h2(s, icc):
            # second halves (stripes 3-4)
            vs, xse, xso = vparts.pop((s, icc))
            nc.vector.tensor_sub(vs[0][:, HNT:NT, 1:W + 1],
                                 xso[:, HNT - 1:NT - 1, :],
                                 xso[:, HNT:NT, :])
            nc.vector.tensor_add(vs[1][:, HNT:, 1:W + 1],
                                 xse[:, HNT:, :], xso[:, HNT:, :])
            nc.vector.tensor_sub(vs[2][:, HNT:, 1:W + 1],
                                 xso[:, HNT:, :], xse[:, HNT:, :])
            # V3 t=27 row: x[54] (d3 is the zero pad row)
            nc.vector.tensor_sub(vs[3][:, HNT:NT - 1, 1:W + 1],
                                 xse[:, HNT:NT - 1, :],
                                 xse[:, HNT + 1:NT, :])
            nc.vector.tensor_copy(vs[3][:, NT - 1:NT, 1:W + 1],
                                  xse[:, NT - 1:NT, :])

        def v_build(s, icc, eng=None, edge_engine=None):
            v_build_h1(s, icc, eng=eng, edge_engine=edge_engine)
            v_build_h2(s, icc)

        def compute_oc(s, oc, warmup=False):
            # bias folds into the M1 drain: exactly one bias reaches each
            # output phase (even = M0+M1'+M2', odd = M1'-M2'-M3 with
            # M1' = 0.5*raw + bias, M2' = 0.5*raw), so the inverse can
            # write f32 output directly -- no separate Act bias pass
            of = of_pool.tile([P, H * W], F32, name=f"of_{s}_{oc}", tag="of")
            ofr = of.rearrange("p (t two w) -> p t two w", two=2, w=W)

            def alloc_accs(t0, t1):
                stw = (t1 - t0) * W
                return [acc_psum.tile([P, stw], F32,
                                      name=f"acc_{s}_{oc}_{t0}_{j}",
                                      tag="acc")
                        for j in range(4)]

            def mm(t0, t1, accs, icc, first, last):
                for kw in range(KK):
                    for j in JORD:
                        rhs = vt[(s, icc)][j][:, t0:t1, kw:kw + W]
                        nc.tensor.matmul(
                            accs[j], stat[(s, icc, oc, j)][kw], rhs,
                            start=(first and kw == 0),
                            stop=(last and kw == KK - 1))

            def finish(t0, t1, accs):
                stw = (t1 - t0) * W
                ms = []
                for j in range(4):
                    mj = m_pool.tile([P, stw], BF16,
                                     name=f"m_{s}_{oc}_{t0}_{j}", tag="m")
                    if j == 1:
                        nc.scalar.activation(
                            mj, accs[j], mybir.ActivationFunctionType.Identity,
                            bias=bias_t[:, oc:oc + 1], scale=0.5)
                    elif j == 2:
                        nc.scalar.mul(mj, accs[j], 0.5)
                    else:
                        nc.scalar.copy(mj, accs[j])
                    ms.append(mj)
                # inverse on DVE; the phase-writing ops emit f32 directly
                tmp = t_pool.tile([P, stw], BF16, name=f"it_{s}_{oc}_{t0}",
                                  tag="it")
                msr = [m.rearrange("p (t w) -> p t w", w=W) for m in ms]
                tmpr = tmp.rearrange("p (t w) -> p t w", w=W)
                nc.vector.tensor_add(tmpr, msr[0], msr[1])
                nc.vector.tensor_add(ofr[:, t0:t1, 0, :], tmpr, msr[2])
                nc.vector.tensor_sub(tmpr, msr[1], msr[2])
                nc.vector.tensor_sub(ofr[:, t0:t1, 1, :], tmpr, msr[3])
                # store this stripe (scalar HWDGE ring)
                lo, hi = t0 * 2 * W, t1 * 2 * W
                nc.scalar.dma_start(out=o_nat[s, oc][:, lo:hi],
                                    in_=of[:, lo:hi])

            if warmup:
                # stripe-pair interleave: both stripes' ic0 matmuls run
                # back-to-back (8 PSUM banks) so ic1's mask/x transfers
                # get ~4.6us of streaming grace behind ic0's
                (a0, a1), (b0, b1) = STRIPES[0], STRIPES[1]
                accsA, accsB = alloc_accs(a0, a1), alloc_accs(b0, b1)
                mm(a0, a1, accsA, 0, True, False)
                mm(b0, b1, accsB, 0, True, False)
                mm(a0, a1, accsA, 1, False, True)
                mm(b0, b1, accsB, 1, False, True)
                finish(a0, a1, accsA)
                finish(b0, b1, accsB)
                rest = STRIPES[2:]
            else:
                rest = STRIPES
            for (t0, t1) in rest:
                accs = alloc_accs(t0, t1)
                mm(t0, t1, accs, 0, True, False)
                mm(t0, t1, accs, 1, False, True)
                finish(t0, t1, accs)

        # --- sample 0 prologue.  Strictly critical-first: the sync ring
        # carries the oc0 mask/weight halves in consumption order, the
        # scalar ring carries the x first-chunks.  Everything else is
        # deferred past the critical window (the DMA engines round-robin
        # ALL queued transfers, so an early trigger steals bandwidth). ---
        nc.sync.dma_start(out=w_tiles[(0, 0)], in_=wt_d[0, 0])
        load_mt(0, 0, 0)
        load_x(0, 0, ring=nc.scalar, split=True)
        nc.sync.dma_start(out=w_tiles[(1, 0)], in_=wt_d[1, 0])
        load_mt(0, 1, 0)
        load_x(0, 1, ring=nc.scalar, split=True)
        # deferred semi-critical: x second chunks (needed by stripe 3,
        # ~9us after the first matmul).  Emitted BEFORE the v_builds that
        # read them -- dependency tracking follows emission order.
        with tc.tile_wait_until(0.010):
            load_x_rest(0, 0, ring=nc.scalar)
            load_x_rest(0, 1, ring=nc.scalar)
        # deferred non-critical: oc1 mask/weight halves, on the sync ring
        # (idle after the critical prologue; the scalar ring carries the
        # stripe stores which would queue these out past +30us)
        with tc.tile_wait_until(0.012):
            nc.sync.dma_start(out=w_tiles[(0, 1)], in_=wt_d[0, 1])
            nc.sync.dma_start(out=w_tiles[(1, 1)], in_=wt_d[1, 1])
            load_mt(0, 0, 1, ring=nc.sync)
            load_mt(0, 1, 1, ring=nc.sync)
        u_build(0, 0, 0)
        v_build_h1(0, 0, edge_engine=nc.gpsimd)
        u_build(0, 1, 0)
        v_build_h1(0, 1, edge_engine=nc.gpsimd)
        v_build_h2(0, 0)
        v_build_h2(0, 1)
        u_build(0, 0, 1)
        u_build(0, 1, 1)
        # bias: 4B-per-descriptor storm -> idle gpsimd ring, deferred out
        # of the critical window (first needed at the first PSUM drain)
        with tc.tile_wait_until(0.008):
            nc.gpsimd.dma_start(out=bias_t,
                                in_=b_d.rearrange("(c p) -> p c", p=P))

        # --- software-pipelined emission: the next sample's ic0 prep sits
        # between this sample's oc0 and oc1 so its first stationaries and V
        # tiles are ready on the DVE before the sample boundary ---
        for s in range(BPC):
            compute_oc(s, 0, warmup=(s == 0))
            if s + 1 < BPC:
                load_mt(s + 1, 0, 0)
                load_mt(s + 1, 1, 0)
                u_build(s + 1, 0, 0)
                load_x(s + 1, 0)
                v_build(s + 1, 0, edge_engine=nc.gpsimd)
            compute_oc(s, 1)
            if s + 1 < BPC:
                u_build(s + 1, 1, 0)
                load_x(s + 1, 1)
                v_build(s + 1, 1, edge_engine=nc.gpsimd)
                load_mt(s + 1, 0, 1)
                load_mt(s + 1, 1, 1)
                u_build(s + 1, 0, 1)
                u_build(s + 1, 1, 1)

    nc.compile()
    return nc


def shard_inputs(x, m, weight, bias):
    x = np.ascontiguousarray(
        np.asarray(x, dtype=np.float32)).astype(ml_dtypes.bfloat16)
    m = np.asarray(m, dtype=np.float32)
    weight = np.asarray(weight, dtype=np.float32)
    bias = np.ascontiguousarray(np.asarray(bias, dtype=np.float32))
    # oc-major mask layout: [B, NI, NO, P_fin, (kh kw o_in)]
    mt = np.ascontiguousarray(
        m.reshape(B, NO, P, NI, P, KK, KK).transpose(0, 3, 1, 4, 5, 6, 2)
    ).reshape(B, NI, NO, P, OCF).astype(ml_dtypes.bfloat16)
    wt = np.ascontiguousarray(
        weight.reshape(NO, P, NI, P, KK, KK).transpose(2, 0, 3, 4, 5, 1)
    ).reshape(NI, NO, P, OCF).astype(ml_dtypes.bfloat16)
    in_maps = []
    for c in range(N_CORES):
        sl = slice(c * BPC, (c + 1) * BPC)
        in_maps.append({"x": x[sl], "mt": mt[sl], "wt": wt, "bias": bias})
    return in_maps


def kernel(x, m, weight, bias, _trace=False):
    nc = build_program()
    in_maps = shard_inputs(x, m, weight, bias)
    res = bass_utils.run_bass_kernel_spmd(
        nc, in_maps, core_ids=list(range(N_CORES)), trace=_trace
    )
    out = np.concatenate([res.results[c]["out"] for c in range(N_CORES)], axis=0)
    if _trace:
        kernel.last_results = res
    return out
